# revision 9
# baseline (speedup 1.0000x reference)
"""VQ codebook kernel for 8 TRN2 NeuronCores.

Data-parallel: z [32,256,32,32] sharded 4 batches/core; codebook [1024,256]
replicated. Per core the device computes, for each of 4096 latent vectors,
psum = 2*z@cb.T - (zsq + csq_q)  (= -d, bit-matching the fp32 reference's
double-rounded distance), then argmin via Max8/MaxIndex with first-index
tie-break, gathers codebook rows for z_q, and transposes back to NCHW.
Host does input layout/splits, an exact recompute of near-tie rows, and the
scalar loss reduction.
"""
import sys

sys.path.insert(0, "/opt/trn_rl_repo")

import numpy as np
import ml_dtypes

B, C, HW, K, D = 32, 256, 1024, 1024, 256
CORES = 8
BPC = B // CORES          # batches per core
TILES = BPC * 8           # 128-row tiles per core
NPC = BPC * HW            # rows per core
BF = ml_dtypes.bfloat16
BINADES = list(range(3, 11))  # zsq binades covered by selector rows
NEX = 3 + len(BINADES)        # extra contract rows: A,B,C + selectors

_CACHE = {}
LAST_RESULTS = None


def _build():
    if "nc" in _CACHE:
        return _CACHE["nc"]
    import concourse.bass as bass
    import concourse.mybir as mybir
    from concourse import tile
    from concourse.masks import make_identity

    f32, bf16, u32 = mybir.dt.float32, mybir.dt.bfloat16, mybir.dt.uint32
    nc = bass.Bass("TRN2", target_bir_lowering=False, debug=False)

    zTh = nc.dram_tensor("zTh", [BPC, 2, 128, HW], bf16, kind="ExternalInput").ap()
    cbh = nc.dram_tensor("cbh", [2, 128, K], bf16, kind="ExternalInput").ap()
    cbl = nc.dram_tensor("cbl", [2, 128, K], bf16, kind="ExternalInput").ap()
    exL = nc.dram_tensor("exL", [16, TILES * 128], bf16, kind="ExternalInput").ap()
    exR = nc.dram_tensor("exR", [16, K], bf16, kind="ExternalInput").ap()
    cbf = nc.dram_tensor("cbf", [K, D], f32, kind="ExternalInput").ap()
    zq_out = nc.dram_tensor("zq_out", [BPC, 2, 128, HW], f32, kind="ExternalOutput").ap()
    aux_out = nc.dram_tensor("aux_out", [96, 128], f32, kind="ExternalOutput").ap()

    with tile.TileContext(nc) as tc:
        with (
            tc.tile_pool(name="const", bufs=1) as cpool,
            tc.tile_pool(name="zpool", bufs=2) as zpool,
            tc.tile_pool(name="work", bufs=4) as work,
            tc.tile_pool(name="outp", bufs=2) as outp,
            tc.tile_pool(name="psmm", bufs=3, space="PSUM") as psmm,
            tc.tile_pool(name="pstr", bufs=1, space="PSUM") as pstr,
        ):
            cbh_t = {}
            cbl_t = {}
            for ch in range(2):
                cbh_t[ch] = cpool.tile([128, K], bf16, name=f"cbh_t{ch}")
                cbl_t[ch] = cpool.tile([128, K], bf16, name=f"cbl_t{ch}")
            exL_t = cpool.tile([16, TILES * 128], bf16, name="exL_t")
            exR_t = cpool.tile([16, K], bf16, name="exR_t")
            ident = cpool.tile([128, 128], f32, name="ident")
            strip = cpool.tile([128, 96], f32, name="strip")
            for ch in range(2):
                nc.sync.dma_start(cbh_t[ch][:, :], cbh[ch, :, :])
                nc.sync.dma_start(cbl_t[ch][:, :], cbl[ch, :, :])
            nc.sync.dma_start(exL_t[:, :], exL[:, :])
            nc.sync.dma_start(exR_t[:, :], exR[:, :])
            make_identity(nc, ident[:, :])

            for b in range(BPC):
                zh_t = {}
                for ch in range(2):
                    zh_t[ch] = zpool.tile([128, HW], bf16, tag=f"zh{ch}", name=f"zh{ch}")
                    nc.sync.dma_start(zh_t[ch][:, :], zTh[b, ch, :, :])
                zqT = {}
                for ch in range(2):
                    zqT[ch] = outp.tile([128, HW], f32, tag=f"zqT{ch}", name=f"zqT{ch}")

                for j in range(8):
                    t = b * 8 + j
                    nsl = slice(j * 128, (j + 1) * 128)
                    esl = slice(t * 128, (t + 1) * 128)
                    psum = psmm.tile([128, K], mybir.dt.float32, tag="psum")
                    dneg = work.tile([128, K], mybir.dt.float32, tag="dneg")
                    for kh in range(2):
                        ksl = slice(kh * 512, (kh + 1) * 512)
                        first = True
                        for lhs, rhs in ((zh_t, cbh_t), (zh_t, cbl_t)):
                            for ch in range(2):
                                nc.tensor.matmul(
                                    psum[:, ksl], lhs[ch][:, nsl], rhs[ch][:, ksl],
                                    start=first, stop=False)
                                first = False
                        nc.tensor.matmul(
                            psum[:, ksl], exL_t[:NEX, esl], exR_t[:NEX, ksl],
                            start=False, stop=True)

                    nc.scalar.copy(dneg[:, :], psum[:, :])
                    top8 = work.tile([128, 8], mybir.dt.float32, tag="top8")
                    idx8 = work.tile([128, 8], u32, tag="idx8")
                    nc.vector.max(out=top8[:, :], in_=dneg[:, :])
                    nc.vector.max_index(idx8[:, :], top8[:, :], dneg[:, :])

                    # strips for aux output (idx as f32, max, second max)
                    nc.vector.tensor_copy(strip[:, t:t + 1], idx8[:, 0:1])
                    nc.scalar.copy(strip[:, 32 + t:33 + t], top8[:, 0:1])
                    nc.scalar.copy(strip[:, 64 + t:65 + t], top8[:, 1:2])

                    # gather codebook rows, transpose to [c, n]
                    import concourse.bass as bass_mod
                    zq_g = work.tile([128, D], mybir.dt.float32, tag="zq_g")
                    nc.gpsimd.indirect_dma_start(
                        out=zq_g[:, :], out_offset=None,
                        in_=cbf[:, :],
                        in_offset=bass_mod.IndirectOffsetOnAxis(ap=idx8[:, 0:1], axis=0),
                    )
                    for ch in range(2):
                        pt = pstr.tile([128, 128], mybir.dt.float32, tag="pt")
                        nc.tensor.transpose(pt[:, :], zq_g[:, ch * 128:(ch + 1) * 128],
                                            ident[:, :])
                        nc.scalar.copy(zqT[ch][:, nsl], pt[:, :])

                for ch in range(2):
                    nc.sync.dma_start(zq_out[b, ch, :, :], zqT[ch][:, :])

            # transpose the three strips in one shot -> aux
            pa = pstr.tile([96, 128], mybir.dt.float32, tag="pa")
            nc.tensor.transpose(pa[:, :], strip[:, :], ident[:, :])
            aux_sb = cpool.tile([96, 128], f32, name="aux_sb")
            nc.scalar.copy(aux_sb[:, :], pa[:, :])
            nc.sync.dma_start(aux_out[:, :], aux_sb[:, :])

    from splitw_inline import split_waits
    split_waits(nc)
    _CACHE["nc"] = nc
    return nc


# --- inline wait-splitting helper (walrus in this container rejects >1
# sem wait per instruction; hoist extras onto preceding NoOps) ---
import types

_splitw_src = '''
import concourse.mybir as mybir

def split_waits(nc, limit=1):
    k = 0
    for bb in nc.main_func.blocks:
        insts = list(bb.instructions)
        out = []
        changed = False
        for ins in insts:
            si = ins.sync_info
            if si and si.on_wait and len(si.on_wait) > limit:
                waits = list(si.on_wait)
                extra, keep = waits[:-limit], waits[-limit:]
                for w in extra:
                    nop = mybir.InstNoOp(name=f"splitw_{k}"); k += 1
                    nop.engine = ins.engine
                    nop.sync_info = mybir.SyncInfo(on_wait=[w], on_update=[])
                    out.append(nop)
                    nc.register_instruction(nop)
                si.on_wait = keep
                changed = True
            out.append(ins)
        if changed:
            bb.instructions = out
    return k
'''
_m = types.ModuleType("splitw_inline")
exec(_splitw_src, _m.__dict__)
sys.modules["splitw_inline"] = _m


def _bf_split(x):
    h = x.astype(BF)
    l = (x - h.astype(np.float32)).astype(BF)
    return h, l


def kernel(z, codebook):
    from concourse.bass_utils import run_bass_kernel_spmd
    global LAST_RESULTS

    z = np.asarray(z, np.float32)
    cb = np.asarray(codebook, np.float32)
    nc = _build()

    # ---- host precompute ----
    cb2T = np.ascontiguousarray((2.0 * cb).T)              # [256, 1024]
    cbhh, cbll = _bf_split(cb2T)
    cbh_np = np.ascontiguousarray(cbhh.reshape(2, 128, K))
    cbl_np = np.ascontiguousarray(cbll.reshape(2, 128, K))

    zh = z.astype(BF).reshape(B, 2, 128, HW)

    z3 = z.reshape(B, C, HW)
    zsq64 = np.einsum("bch,bch->bh", z3.astype(np.float64), z3.astype(np.float64))
    zsq = zsq64.astype(np.float32).reshape(-1)              # [N], n = b*1024+hw
    zsq64 = zsq64.reshape(-1)

    csq64 = np.sum(cb.astype(np.float64) ** 2, axis=1)
    csq = csq64.astype(np.float32)                          # [K]
    row_e = np.floor(np.log2(zsq.astype(np.float64))).astype(int)
    assert row_e.min() >= BINADES[0] and row_e.max() <= BINADES[-1], (
        row_e.min(), row_e.max())
    csq_q = {}
    for e in BINADES:
        Q = 2.0 ** (e - 23)
        csq_q[e] = (np.round(csq.astype(np.float64) / Q) * Q).astype(np.float32)

    # zsq 3-way bf16 split (exact)
    A = zsq.astype(BF).astype(np.float32)
    r1 = zsq - A
    Bs = r1.astype(BF).astype(np.float32)
    Cs = (r1 - Bs)
    exL_np = np.zeros((16, B * HW), BF)
    exL_np[0] = A.astype(BF)
    exL_np[1] = Bs.astype(BF)
    exL_np[2] = Cs.astype(BF)
    for i, e in enumerate(BINADES):
        exL_np[3 + i] = (row_e == e).astype(np.float32).astype(BF)
    exR_np = np.zeros((16, K), BF)
    exR_np[0] = exR_np[1] = exR_np[2] = np.float32(-1.0).astype(BF)
    for i, e in enumerate(BINADES):
        exR_np[3 + i] = (-csq_q[e]).astype(BF)
        assert np.all(exR_np[3 + i].astype(np.float32) == -csq_q[e])

    in_maps = []
    for c in range(CORES):
        bsl = slice(c * BPC, (c + 1) * BPC)
        nsl = slice(c * NPC, (c + 1) * NPC)
        in_maps.append({
            "zTh": np.ascontiguousarray(zh[bsl]),
            "cbh": cbh_np, "cbl": cbl_np,
            "exL": np.ascontiguousarray(exL_np[:, nsl]),
            "exR": exR_np,
            "cbf": cb,
        })

    res = run_bass_kernel_spmd(nc, in_maps, core_ids=list(range(CORES)))
    LAST_RESULTS = res

    # ---- assemble ----
    N = B * HW
    zq = np.empty((B, C, HW), np.float32)
    idx = np.empty(N, np.int64)
    m0 = np.empty(N, np.float64)
    for c in range(CORES):
        r = res.results[c]
        zq[c * BPC:(c + 1) * BPC] = r["zq_out"].reshape(BPC, C, HW)
        aux = r["aux_out"]
        nsl = slice(c * NPC, (c + 1) * NPC)
        idx[nsl] = aux[0:32].reshape(-1).astype(np.int64)
        m0[nsl] = aux[32:64].reshape(-1).astype(np.float64)
        m1 = aux[64:96].reshape(-1).astype(np.float64)
        if c == 0:
            gap_all = np.empty(N, np.float64)
        gap_all[nsl] = m0[nsl] - m1

    # ---- host fixup of near-tie rows (exact reference emulation) ----
    zf = z3.transpose(0, 2, 1).reshape(N, C)               # [N, 256] row n
    Q_row = np.power(2.0, row_e - 23)
    suspect = gap_all <= (2.0 * Q_row + 4.2e-4)
    sus = np.nonzero(suspect)[0]
    if len(sus):
        cb2T64 = (2.0 * cb).astype(np.float64).T
        mmv = zf[sus].astype(np.float64) @ cb2T64           # [S, K]
        psum32 = mmv.astype(np.float32)
        for e in np.unique(row_e[sus]):
            rows = sus[row_e[sus] == e]
            rl = np.searchsorted(sus, rows)
            t1 = zsq[rows].astype(np.float64)[:, None] + \
                csq_q[e].astype(np.float64)[None, :]
            dneg_s = (psum32[rl].astype(np.float64) - t1).astype(np.float32)
            idx_new = np.argmax(dneg_s, axis=1)
            m_new = dneg_s[np.arange(len(rl)), idx_new].astype(np.float64)
            idx[rows] = idx_new
            m0[rows] = m_new
        # patch zq for changed rows
        bb, hh = np.divmod(sus, HW)
        zq[bb, :, hh] = cb[idx[sus]]

    # ---- loss (validated identity) ----
    csq_q_sel = np.stack([csq_q[e] for e in BINADES])[
        np.searchsorted(BINADES, row_e)]
    dcsq = csq.astype(np.float64)[idx] - \
        csq_q_sel[np.arange(N), idx].astype(np.float64)
    loss_sum = np.sum(-m0 + dcsq + (zsq64 - zsq.astype(np.float64)))
    loss = np.float32(1.25 * loss_sum / (N * D))

    zq = zq.reshape(B, C, 32, 32)
    return zq, idx.astype(np.int32), loss
